# revision 1
# baseline (speedup 1.0000x reference)
"""Self-contained TRN2 Bass kernel for the CustomMaskRCNN mask-loss problem.

kernel(**inputs) takes the FULL unsharded inputs (mask_logits [512,2,28,28],
proposals [512,4], gt_boxes [200,4], gt_masks [200,520,704], gt_labels [200])
and returns the scalar float32 loss, computed data-parallel over proposals on
8 NeuronCores (64 proposals per core):
  IoU + argmax match on the vector engine; integer box clamp; matched-mask
  windows fetched by indirect DMA (one 2-row run per SBUF partition, two
  proposals per call); bilinear 28x28 resize done as PE matmuls against
  "hat"-function interpolation matrices (relu(1-|k-s|)), which reproduces
  the reference's align_corners=False bilinear exactly; masked BCE partial
  sums reduced on-chip.  Host sums the 8 (bce_sum, num_pos) pairs into the
  global mean.
"""
import os
import sys
import types

sys.path.insert(0, "/opt/trn_rl_repo")

import numpy as np
from contextlib import ExitStack

import concourse.bass as bass
import concourse.tile as tile
from concourse import mybir
from concourse.bass_utils import run_bass_kernel_spmd

# ---------------------------------------------------------------------------
# compatibility patches for this container's neuronxcc build
# ---------------------------------------------------------------------------


MAX_WAITS = 1
_applied = [False]


def apply_patches():
    if _applied[0]:
        return
    _applied[0] = True

    def _patched_cafs(self, sems):
        if not sems:
            return
        sem_nums = [s.num if hasattr(s, "num") else s for s in sems]
        for r in bass.compact_to_ranges(sem_nums):
            assert self._state.free_isdisjoint(r)
            self.gpsimd.dma_reset(r)  # drain w/ is_reset_sema resets the range
        self._state.prepend_free_semaphores(sem_nums)
        for poison_set in self._tile_sem_poison_stack:
            poison_set.update(sem_nums)

    bass.Bass.clear_and_free_semaphores = _patched_cafs


def split_excess_waits(nc):
    ctr = [0]
    for fn in nc.m.functions:
        for blk in fn.blocks:
            insts = list(blk.instructions)
            out = []
            changed = False
            for ins in insts:
                si = getattr(ins, "sync_info", None)
                if si is not None and si.on_wait and len(si.on_wait) > MAX_WAITS:
                    waits = list(si.on_wait)
                    excess, keep = waits[:-MAX_WAITS], waits[-MAX_WAITS:]
                    while excess:
                        chunk, excess = excess[:MAX_WAITS], excess[MAX_WAITS:]
                        ctr[0] += 1
                        out.append(mybir.InstNoOp(
                            name=f"I-waitsplit-{ctr[0]}",
                            engine=ins.engine,
                            bass_nofuse=True,
                            sync_info=mybir.SyncInfo(on_wait=chunk, on_update=[]),
                        ))
                    si.on_wait = keep
                    changed = True
                out.append(ins)
            if changed:
                blk.instructions = out
    return ctr[0]



F32 = mybir.dt.float32
I32 = mybir.dt.int32
AF = mybir.ActivationFunctionType
OP = mybir.AluOpType

P, G, H, W = 512, 200, 520, 704
PC = 64          # proposals per core
M = 28           # mask size
HW = H * W
SEG = 128        # crop window (rows and cols)

# engines used to issue the 64 crop DMAs (spread the SWDGE issue cost)
def _crop_engines(nc):
    return [nc.sync, nc.gpsimd, nc.scalar, nc.vector]


def _floor_seq(nc, pool, x_ap, shape, name):
    """Exact floor for x>=0 on HW (f32->i32 conversion rounds to nearest).
    Returns (floor_f32_tile, floor_i32_tile)."""
    fi = pool.tile(shape, I32, tag=f"{name}_fi")
    nc.vector.tensor_copy(out=fi[:], in_=x_ap)
    ff = pool.tile(shape, F32, tag=f"{name}_ff")
    nc.vector.tensor_copy(out=ff[:], in_=fi[:])
    gt = pool.tile(shape, F32, tag=f"{name}_gt")
    nc.vector.tensor_sub(out=gt[:], in0=ff[:], in1=x_ap)
    nc.vector.tensor_scalar(out=gt[:], in0=gt[:], scalar1=0.0, scalar2=None,
                            op0=OP.is_gt)
    nc.vector.tensor_sub(out=ff[:], in0=ff[:], in1=gt[:])
    fi2 = pool.tile(shape, I32, tag=f"{name}_fi2")
    nc.vector.tensor_copy(out=fi2[:], in_=ff[:])
    return ff, fi2


def build(nc: bass.Bass):
    logitsT = nc.dram_tensor("logitsT", [M, PC * M], F32, kind="ExternalInput")
    props = nc.dram_tensor("props", [PC, 4], F32, kind="ExternalInput")
    gtbr = nc.dram_tensor("gtbr", [PC, 4 * G], F32, kind="ExternalInput")
    masksflat = nc.dram_tensor("masksflat", [G * HW + W], F32, kind="ExternalInput")
    out = nc.dram_tensor("out", [2], F32, kind="ExternalOutput")
    scr_sy = nc.dram_tensor("scr_sy", [PC, M], F32)
    scr_sx = nc.dram_tensor("scr_sx", [PC, M], F32)
    scr_pos = nc.dram_tensor("scr_pos", [PC, 1], F32)
    scr_midx = nc.dram_tensor("scr_midx", [PC, 1], F32)
    scr_p2 = nc.dram_tensor("scr_p2", [PC, 1], F32)

    with tile.TileContext(nc) as tc, ExitStack() as ctx:
        pool = ctx.enter_context(tc.tile_pool(name="sbuf", bufs=1))
        crop_pool = ctx.enter_context(tc.tile_pool(name="crops", bufs=8))
        psum = ctx.enter_context(tc.tile_pool(name="psum", bufs=3, space="PSUM"))
        psum_mm = ctx.enter_context(tc.tile_pool(name="psum_mm", bufs=3, space="PSUM"))
        psum_bc = ctx.enter_context(tc.tile_pool(name="psum_bc", bufs=1, space="PSUM"))

        # ---------- constants ----------
        iota_g_i = pool.tile([PC, G], I32)
        nc.gpsimd.iota(iota_g_i[:], pattern=[[1, G]], base=0, channel_multiplier=0)
        iota_g = pool.tile([PC, G], F32)
        nc.vector.tensor_copy(out=iota_g[:], in_=iota_g_i[:])
        iotap_i = pool.tile([128, 1], I32)
        nc.gpsimd.iota(iotap_i[:], pattern=[[0, 1]], base=0, channel_multiplier=1)
        iotap = pool.tile([128, 1], F32)
        nc.vector.tensor_copy(out=iotap[:], in_=iotap_i[:])
        grid_i = pool.tile([PC, M], I32)
        nc.gpsimd.iota(grid_i[:], pattern=[[1, M]], base=0, channel_multiplier=0)
        grid = pool.tile([PC, M], F32)
        nc.vector.tensor_copy(out=grid[:], in_=grid_i[:])
        nc.vector.tensor_scalar_add(out=grid[:], in0=grid[:], scalar1=0.5)
        ones1 = pool.tile([1, 128], F32)
        nc.vector.memset(ones1[:], 1.0)
        ge64_i = pool.tile([128, 1], I32)
        nc.vector.tensor_scalar(out=ge64_i[:], in0=iotap_i[:], scalar1=64,
                                scalar2=None, op0=OP.is_ge)
        iotamod = pool.tile([128, 1], I32)
        nc.vector.tensor_scalar_mul(out=iotamod[:], in0=ge64_i[:], scalar1=-64)
        nc.vector.tensor_add(out=iotamod[:], in0=iotamod[:], in1=iotap_i[:])
        rowv_e = pool.tile([128, 1], F32)
        nc.vector.tensor_copy(out=rowv_e[:], in_=iotamod[:])
        nc.vector.tensor_scalar_mul(out=rowv_e[:], in0=rowv_e[:], scalar1=2.0)
        rowv_o = pool.tile([128, 1], F32)
        nc.vector.tensor_scalar_add(out=rowv_o[:], in0=rowv_e[:], scalar1=1.0)
        ge1m_i = pool.tile([128, 1], I32)
        nc.vector.tensor_scalar(out=ge1m_i[:], in0=ge64_i[:], scalar1=-1,
                                scalar2=1, op0=OP.mult, op1=OP.add)
        ones28 = pool.tile([M, 1], F32)
        nc.vector.memset(ones28[:], 1.0)

        # ---------- inputs ----------
        pr = pool.tile([PC, 4], F32)
        nc.sync.dma_start(out=pr[:], in_=props.ap())
        gb = pool.tile([PC, 4 * G], F32)
        nc.sync.dma_start(out=gb[:], in_=gtbr.ap())
        lg = pool.tile([M, PC * M], F32)
        nc.sync.dma_start(out=lg[:], in_=logitsT.ap())

        px1, py1, px2, py2 = (pr[:, i:i + 1] for i in range(4))
        gx1, gy1, gx2, gy2 = (gb[:, i * G:(i + 1) * G] for i in range(4))

        # ---------- stage 1: IoU [PC, G] ----------
        ltx = pool.tile([PC, G], F32)
        nc.vector.tensor_scalar_max(out=ltx[:], in0=gx1, scalar1=px1)
        iw = pool.tile([PC, G], F32)
        nc.vector.scalar_tensor_tensor(out=iw[:], in0=gx2, scalar=px2, in1=ltx[:],
                                       op0=OP.min, op1=OP.subtract)
        nc.vector.tensor_scalar_max(out=iw[:], in0=iw[:], scalar1=0.0)
        lty = pool.tile([PC, G], F32)
        nc.vector.tensor_scalar_max(out=lty[:], in0=gy1, scalar1=py1)
        ih = pool.tile([PC, G], F32)
        nc.vector.scalar_tensor_tensor(out=ih[:], in0=gy2, scalar=py2, in1=lty[:],
                                       op0=OP.min, op1=OP.subtract)
        nc.vector.tensor_scalar_max(out=ih[:], in0=ih[:], scalar1=0.0)
        inter = pool.tile([PC, G], F32)
        nc.vector.tensor_mul(out=inter[:], in0=iw[:], in1=ih[:])
        aw = pool.tile([PC, 1], F32)
        nc.vector.tensor_sub(out=aw[:], in0=px2, in1=px1)
        ah = pool.tile([PC, 1], F32)
        nc.vector.tensor_sub(out=ah[:], in0=py2, in1=py1)
        area_a = pool.tile([PC, 1], F32)
        nc.vector.tensor_mul(out=area_a[:], in0=aw[:], in1=ah[:])
        gw = pool.tile([PC, G], F32)
        nc.vector.tensor_sub(out=gw[:], in0=gx2, in1=gx1)
        gh = pool.tile([PC, G], F32)
        nc.vector.tensor_sub(out=gh[:], in0=gy2, in1=gy1)
        area_g = pool.tile([PC, G], F32)
        nc.vector.tensor_mul(out=area_g[:], in0=gw[:], in1=gh[:])
        denom = pool.tile([PC, G], F32)
        nc.vector.scalar_tensor_tensor(out=denom[:], in0=area_g[:], scalar=area_a[:],
                                       in1=inter[:], op0=OP.add, op1=OP.subtract)
        rec = pool.tile([PC, G], F32)
        nc.scalar.activation(out=rec[:], in_=denom[:], func=AF.Ln)
        nc.scalar.activation(out=rec[:], in_=rec[:], func=AF.Exp, scale=-1.0)
        iou = pool.tile([PC, G], F32)
        nc.vector.tensor_mul(out=iou[:], in0=inter[:], in1=rec[:])

        # ---------- stage 2: match ----------
        miou = pool.tile([PC, 1], F32)
        nc.vector.tensor_reduce(out=miou[:], in_=iou[:], axis=mybir.AxisListType.X,
                                op=OP.max)
        pos = pool.tile([PC, 1], F32)
        nc.vector.tensor_scalar(out=pos[:], in0=miou[:], scalar1=0.3, scalar2=None,
                                op0=OP.is_gt)
        eq = pool.tile([PC, G], F32)
        nc.vector.tensor_scalar(out=eq[:], in0=iou[:], scalar1=miou[:], scalar2=None,
                                op0=OP.is_ge)
        masked = pool.tile([PC, G], F32)
        nc.vector.scalar_tensor_tensor(out=masked[:], in0=eq[:], scalar=-1024.0,
                                       in1=iota_g[:], op0=OP.mult, op1=OP.add)
        midxf = pool.tile([PC, 1], F32)
        nc.vector.tensor_reduce(out=midxf[:], in_=masked[:], axis=mybir.AxisListType.X,
                                op=OP.min)
        nc.vector.tensor_scalar_add(out=midxf[:], in0=midxf[:], scalar1=1024.0)
        onehot = pool.tile([PC, G], F32)
        nc.vector.tensor_scalar(out=onehot[:], in0=iota_g[:], scalar1=midxf[:],
                                scalar2=None, op0=OP.is_equal)
        mscr = pool.tile([PC, G], F32)
        mb = pool.tile([PC, 4], F32)
        for c, gcomp in enumerate((gx1, gy1, gx2, gy2)):
            nc.vector.tensor_mul(out=mscr[:], in0=onehot[:], in1=gcomp)
            nc.vector.tensor_reduce(out=mb[:, c:c + 1], in_=mscr[:],
                                    axis=mybir.AxisListType.X, op=OP.add)
        midx_i = pool.tile([PC, 1], I32)
        nc.vector.tensor_copy(out=midx_i[:], in_=midxf[:])  # exact integer value

        # ---------- stage 3: crop params ----------
        _, bi = _floor_seq(nc, pool, mb[:], [PC, 4], "bi")   # trunc(mb), mb>=0
        x1c = pool.tile([PC, 1], I32)
        nc.vector.tensor_scalar(out=x1c[:], in0=bi[:, 0:1], scalar1=0, scalar2=W - 1,
                                op0=OP.max, op1=OP.min)
        y1c = pool.tile([PC, 1], I32)
        nc.vector.tensor_scalar(out=y1c[:], in0=bi[:, 1:2], scalar1=0, scalar2=H - 1,
                                op0=OP.max, op1=OP.min)
        x2t = pool.tile([PC, 1], I32)
        nc.vector.tensor_scalar(out=x2t[:], in0=bi[:, 2:3], scalar1=W, scalar2=None,
                                op0=OP.min)
        x1p1 = pool.tile([PC, 1], I32)
        nc.vector.tensor_scalar_add(out=x1p1[:], in0=x1c[:], scalar1=1)
        x2c = pool.tile([PC, 1], I32)
        nc.vector.tensor_max(out=x2c[:], in0=x1p1[:], in1=x2t[:])
        y2t = pool.tile([PC, 1], I32)
        nc.vector.tensor_scalar(out=y2t[:], in0=bi[:, 3:4], scalar1=H, scalar2=None,
                                op0=OP.min)
        y1p1 = pool.tile([PC, 1], I32)
        nc.vector.tensor_scalar_add(out=y1p1[:], in0=y1c[:], scalar1=1)
        y2c = pool.tile([PC, 1], I32)
        nc.vector.tensor_max(out=y2c[:], in0=y1p1[:], in1=y2t[:])
        cw_i = pool.tile([PC, 1], I32)
        nc.vector.tensor_sub(out=cw_i[:], in0=x2c[:], in1=x1c[:])
        ch_i = pool.tile([PC, 1], I32)
        nc.vector.tensor_sub(out=ch_i[:], in0=y2c[:], in1=y1c[:])
        cw_f = pool.tile([PC, 1], F32)
        nc.vector.tensor_copy(out=cw_f[:], in_=cw_i[:])
        ch_f = pool.tile([PC, 1], F32)
        nc.vector.tensor_copy(out=ch_f[:], in_=ch_i[:])
        ox = pool.tile([PC, 1], I32)
        nc.vector.tensor_scalar(out=ox[:], in0=x1c[:], scalar1=W - SEG, scalar2=None,
                                op0=OP.min)
        oy = pool.tile([PC, 1], I32)
        nc.vector.tensor_scalar(out=oy[:], in0=y1c[:], scalar1=H - SEG, scalar2=None,
                                op0=OP.min)
        dx_i = pool.tile([PC, 1], I32)
        nc.vector.tensor_sub(out=dx_i[:], in0=x1c[:], in1=ox[:])
        dx_f = pool.tile([PC, 1], F32)
        nc.vector.tensor_copy(out=dx_f[:], in_=dx_i[:])
        dy_i = pool.tile([PC, 1], I32)
        nc.vector.tensor_sub(out=dy_i[:], in0=y1c[:], in1=oy[:])
        dy_f = pool.tile([PC, 1], F32)
        nc.vector.tensor_copy(out=dy_f[:], in_=dy_i[:])
        # crop base offset, split in two f32-exact parts for partition bcast:
        # part2 = oy*W + ox  (< 2^19, f32-exact); midx broadcast separately
        oyw = pool.tile([PC, 1], I32)
        nc.vector.tensor_scalar_mul(out=oyw[:], in0=oy[:], scalar1=W)
        nc.vector.tensor_add(out=oyw[:], in0=oyw[:], in1=ox[:])
        part2f = pool.tile([PC, 1], F32)
        nc.vector.tensor_copy(out=part2f[:], in_=oyw[:])

        # ---------- stage 4: sample coords (crop-local, continuous) ----------
        def coords(cf, df, scr_dram, name):
            cm1 = pool.tile([PC, 1], F32, tag=f"{name}_cm1")
            nc.vector.tensor_scalar_add(out=cm1[:], in0=cf[:], scalar1=-1.0)
            cd = pool.tile([PC, 1], F32, tag=f"{name}_cd")
            nc.vector.tensor_scalar_mul(out=cd[:], in0=cf[:], scalar1=1.0 / M)
            s = pool.tile([PC, M], F32, tag=f"{name}_s")
            nc.vector.tensor_scalar(out=s[:], in0=grid[:], scalar1=cd[:],
                                    scalar2=-0.5, op0=OP.mult, op1=OP.add)
            nc.vector.tensor_scalar(out=s[:], in0=s[:], scalar1=0.0, scalar2=cm1[:],
                                    op0=OP.max, op1=OP.min)
            nc.vector.tensor_scalar_add(out=s[:], in0=s[:], scalar1=df[:])
            nc.sync.dma_start(out=scr_dram.ap(), in_=s[:])
            flat = pool.tile([1, PC * M], F32, tag=f"{name}_flat")
            nc.sync.dma_start(
                out=flat[:], in_=scr_dram.ap().rearrange("a b -> (a b)").unsqueeze(0))
            return flat

        syflat = coords(ch_f, dy_f, scr_sy, "sy")
        sxflat = coords(cw_f, dx_f, scr_sx, "sx")
        nc.sync.dma_start(out=scr_pos.ap(), in_=pos[:])
        pos_row = pool.tile([1, PC], F32)
        nc.sync.dma_start(out=pos_row[:],
                          in_=scr_pos.ap().rearrange("a b -> (a b)").unsqueeze(0))
        nc.sync.dma_start(out=scr_midx.ap(), in_=midxf[:])
        midx_row = pool.tile([1, PC], F32)
        nc.sync.dma_start(out=midx_row[:],
                          in_=scr_midx.ap().rearrange("a b -> (a b)").unsqueeze(0))
        nc.sync.dma_start(out=scr_p2.ap(), in_=part2f[:])
        p2_row = pool.tile([1, PC], F32)
        nc.sync.dma_start(out=p2_row[:],
                          in_=scr_p2.ap().rearrange("a b -> (a b)").unsqueeze(0))

        # ---------- stage 5: hat interp matrices RyT/RxT [128, PC*M] ----------
        def hat_matrix(flat, name, iotavec):
            CH = 448
            dmat = pool.tile([128, PC * M], F32, tag="hat_dmat")
            for c in range(4):
                bps = psum_bc.tile([128, CH], F32, tag="bc")
                nc.tensor.matmul(out=bps[:], lhsT=ones1[:],
                                 rhs=flat[:, c * CH:(c + 1) * CH],
                                 start=True, stop=True)
                nc.vector.tensor_tensor(out=dmat[:, c * CH:(c + 1) * CH],
                                        in0=iotavec[:].to_broadcast([128, CH]),
                                        in1=bps[:], op=OP.subtract)
            habs = pool.tile([128, PC * M], F32, tag="hat_habs")
            nc.scalar.activation(out=habs[:], in_=dmat[:], func=AF.Abs)
            rt = pool.tile([128, PC * M], F32, tag=f"{name}_rt")
            nc.scalar.activation(out=rt[:], in_=habs[:], func=AF.Relu,
                                 scale=-1.0, bias=1.0)
            return rt

        ryt_e = hat_matrix(syflat, "rye", rowv_e)
        ryt_o = hat_matrix(syflat, "ryo", rowv_o)
        rxt = hat_matrix(sxflat, "rx", iotap)

        # ---------- stage 6: crop row offsets + indirect crop gathers ----------
        mbc_ps = psum_bc.tile([128, PC], F32, tag="bc")
        nc.tensor.matmul(out=mbc_ps[:], lhsT=ones1[:], rhs=midx_row[:],
                         start=True, stop=True)
        idx_crop = pool.tile([128, PC], I32)
        nc.vector.tensor_copy(out=idx_crop[:], in_=mbc_ps[:])
        nc.vector.tensor_scalar_mul(out=idx_crop[:], in0=idx_crop[:],
                                    scalar1=HW // 128)
        nc.vector.tensor_scalar(out=idx_crop[:], in0=idx_crop[:], scalar1=7,
                                scalar2=None, op0=OP.arith_shift_left)
        p2c_ps = psum_bc.tile([128, PC], F32, tag="bc")
        nc.tensor.matmul(out=p2c_ps[:], lhsT=ones1[:], rhs=p2_row[:],
                         start=True, stop=True)
        p2i = pool.tile([128, PC], I32)
        nc.vector.tensor_copy(out=p2i[:], in_=p2c_ps[:])
        nc.vector.tensor_add(out=idx_crop[:], in0=idx_crop[:], in1=p2i[:])
        rowoff = pool.tile([128, 1], I32)
        nc.vector.tensor_scalar_mul(out=rowoff[:], in0=iotamod[:], scalar1=2 * W)
        nc.vector.tensor_tensor(out=idx_crop[:], in0=idx_crop[:],
                                in1=rowoff[:].to_broadcast([128, PC]), op=OP.add)
        # per-call index column: top half -> even proposal, bottom half -> odd
        idx2 = pool.tile([128, PC // 2], I32)
        idx_v = idx_crop[:].rearrange("q (j t) -> q j t", t=2)
        nc.vector.tensor_tensor(out=idx2[:], in0=idx_v[:, :, 0],
                                in1=ge1m_i[:].to_broadcast([128, PC // 2]),
                                op=OP.mult)
        scr2 = pool.tile([128, PC // 2], I32)
        nc.vector.tensor_tensor(out=scr2[:], in0=idx_v[:, :, 1],
                                in1=ge64_i[:].to_broadcast([128, PC // 2]),
                                op=OP.mult)
        nc.vector.tensor_add(out=idx2[:], in0=idx2[:], in1=scr2[:])

        targets = pool.tile([M, PC * M], F32)
        masks2d = masksflat.ap().unsqueeze(1)
        RUN = W + SEG  # 2-row run: row r cols ox.. plus row r+1 window at +W
        for j in range(PC // 2):
            crop = crop_pool.tile([SEG, RUN], F32, tag="crop")
            nc.gpsimd.indirect_dma_start(
                out=crop[:], out_offset=None, in_=masks2d,
                in_offset=bass.IndirectOffsetOnAxis(ap=idx2[:, j:j + 1], axis=0),
            )
            for p, qb in ((2 * j, 0), (2 * j + 1, 64)):
                t1t_ps = psum.tile([SEG, M], F32, tag="t1t")
                nc.tensor.matmul(out=t1t_ps[:],
                                 lhsT=crop[qb:qb + 64, 0:SEG],
                                 rhs=ryt_e[qb:qb + 64, p * M:(p + 1) * M],
                                 start=True, stop=False)
                nc.tensor.matmul(out=t1t_ps[:],
                                 lhsT=crop[qb:qb + 64, W:W + SEG],
                                 rhs=ryt_o[qb:qb + 64, p * M:(p + 1) * M],
                                 start=False, stop=True)
                t1t = crop_pool.tile([SEG, M], F32, tag="t1tsb")
                nc.scalar.copy(out=t1t[:], in_=t1t_ps[:])
                tg_ps = psum_mm.tile([M, M], F32, tag="tg")
                nc.tensor.matmul(out=tg_ps[:], lhsT=rxt[:, p * M:(p + 1) * M],
                                 rhs=t1t[:], start=True, stop=True)
                nc.scalar.copy(out=targets[:, p * M:(p + 1) * M], in_=tg_ps[:])

        # ---------- stage 7: masked BCE ----------
        posbc_ps = psum_bc.tile([M, PC], F32, tag="bc")
        nc.tensor.matmul(out=posbc_ps[:], lhsT=ones1[0:1, 0:M], rhs=pos_row[:],
                         start=True, stop=True)
        pos_bc = pool.tile([M, PC], F32)
        nc.scalar.copy(out=pos_bc[:], in_=posbc_ps[:])
        lm = pool.tile([M, PC * M], F32)
        nc.vector.tensor_tensor(
            out=lm[:].rearrange("n (p m) -> n p m", p=PC),
            in0=lg[:].rearrange("n (p m) -> n p m", p=PC),
            in1=pos_bc[:].unsqueeze(2).to_broadcast([M, PC, M]),
            op=OP.mult)
        scr = pool.tile([M, PC * M], F32)
        cross = pool.tile([M, 1], F32)
        nc.vector.tensor_mul(out=scr[:], in0=lm[:], in1=targets[:])
        nc.vector.tensor_reduce(out=cross[:], in_=scr[:],
                                axis=mybir.AxisListType.X, op=OP.add)
        sabs = pool.tile([M, PC * M], F32)
        nc.scalar.activation(out=sabs[:], in_=lg[:], func=AF.Abs)
        nc.scalar.activation(out=sabs[:], in_=sabs[:], func=AF.Exp, scale=-1.0)
        nc.scalar.activation(out=sabs[:], in_=sabs[:], func=AF.Ln, bias=1.0)
        srelu = pool.tile([M, PC * M], F32)
        nc.scalar.activation(out=srelu[:], in_=lg[:], func=AF.Relu)
        sp = pool.tile([M, PC * M], F32)
        nc.vector.tensor_add(out=sp[:], in0=sabs[:], in1=srelu[:])
        spm = pool.tile([M, 1], F32)
        nc.vector.tensor_tensor(
            out=scr[:].rearrange("n (p m) -> n p m", p=PC),
            in0=sp[:].rearrange("n (p m) -> n p m", p=PC),
            in1=pos_bc[:].unsqueeze(2).to_broadcast([M, PC, M]),
            op=OP.mult)
        nc.vector.tensor_reduce(out=spm[:], in_=scr[:],
                                axis=mybir.AxisListType.X, op=OP.add)
        bce_col = pool.tile([M, 1], F32)
        nc.vector.tensor_sub(out=bce_col[:], in0=spm[:], in1=cross[:])
        tot_ps = psum_bc.tile([1, 1], F32, tag="bc")
        nc.tensor.matmul(out=tot_ps[:], lhsT=ones28[:], rhs=bce_col[:],
                         start=True, stop=True)
        out_sb = pool.tile([1, 2], F32)
        nc.scalar.copy(out=out_sb[:, 0:1], in_=tot_ps[:])
        nc.vector.tensor_reduce(out=out_sb[:, 1:2], in_=pos_row[:],
                                axis=mybir.AxisListType.X, op=OP.add)
        nc.sync.dma_start(out=out.ap().unsqueeze(0), in_=out_sb[:])

    return nc


def prep_inputs(mask_logits, proposals, gt_boxes, gt_masks, gt_labels=None):
    """Full inputs -> list of 8 per-core input maps."""
    mask_logits = np.asarray(mask_logits, np.float32)
    proposals = np.asarray(proposals, np.float32)
    gt_boxes = np.asarray(gt_boxes, np.float32)
    gt_masks = np.asarray(gt_masks, np.float32)
    gtbr = np.tile(gt_boxes.T.reshape(1, 4 * G), (PC, 1)).astype(np.float32)
    gtbr = np.ascontiguousarray(gtbr)
    masksflat = np.concatenate([gt_masks.reshape(-1), np.zeros(W, np.float32)])
    maps = []
    for c in range(8):
        sl = slice(c * PC, (c + 1) * PC)
        L = mask_logits[sl, 1]                      # [PC, M(m=y), M(n=x)]
        logitsT = np.ascontiguousarray(L.transpose(2, 0, 1).reshape(M, PC * M))
        maps.append({
            "logitsT": logitsT,
            "props": np.ascontiguousarray(proposals[sl]),
            "gtbr": gtbr,
            "masksflat": masksflat,
        })
    return maps


def combine_outputs(outs):
    """outs: list of 8 np arrays [2] -> scalar float32 loss."""
    s = np.float32(0.0)
    n = np.float32(0.0)
    for o in outs:
        s = np.float32(s + np.float32(o[0]))
        n = np.float32(n + np.float32(o[1]))
    denom = np.float32(max(n, np.float32(1.0)) * np.float32(M * M))
    loss = np.float32(s / denom)
    return np.float32(loss if n > 0 else 0.0)


# ---------------------------------------------------------------------------
# public entry point
# ---------------------------------------------------------------------------
LAST_EXEC_NS = None
_BUILT = None


def _get_program():
    global _BUILT
    if _BUILT is None:
        apply_patches()
        nc = bass.Bass("TRN2", debug=False)
        build(nc)
        split_excess_waits(nc)
        _BUILT = nc
    return _BUILT


def kernel(mask_logits, proposals, gt_boxes, gt_masks, gt_labels=None, **_):
    global LAST_EXEC_NS
    nc = _get_program()
    maps = prep_inputs(mask_logits, proposals, gt_boxes, gt_masks, gt_labels)
    trace = os.environ.get("BASSKERNEL_TRACE", "0") == "1"
    if trace:
        try:
            from trn_agent_boot.trn_boot import _ntff_profile_via_ctypes
            hook = _ntff_profile_via_ctypes("/opt/axon/libaxon_pjrt.so")
            m = types.ModuleType("antenv.axon_hooks")
            m.get_axon_ntff_profile_hook = lambda: hook
            sys.modules["antenv.axon_hooks"] = m
        except Exception:
            trace = False
    res = run_bass_kernel_spmd(nc, maps, core_ids=list(range(8)), trace=trace)
    LAST_EXEC_NS = res.exec_time_ns
    outs = [res.results[c]["out"] for c in range(8)]
    return combine_outputs(outs)



# revision 17
# speedup vs baseline: 1.6090x; 1.6090x over previous
"""Self-contained TRN2 Bass kernel for the CustomMaskRCNN mask-loss problem.

kernel(**inputs) takes the FULL unsharded inputs (mask_logits [512,2,28,28],
proposals [512,4], gt_boxes [200,4], gt_masks [200,520,704], gt_labels [200])
and returns the scalar float32 loss, computed data-parallel over proposals on
8 NeuronCores (64 proposals per core).

v2 design (PE-bound rework of the fp32 baseline):
  - gt_masks shipped as bf16 rows [G*H+8, W]; each proposal's 128x128 crop is
    fetched with ONE direct DMA descriptor ([[W,128],[1,128]] pattern) whose
    DRAM offset is a runtime register (values_load of an on-chip computed
    offset) -- no gpsimd indirect-DMA descriptor generation at all.
  - bilinear resize as bf16 PE matmuls against hat interpolation matrices
    relu(1-|q-s|), built by a 3-row bf16 matmul (iota row, ones, ones) x
    (ones, -floor(s), -frac(s)) so the subtraction happens in fp32 PSUM.
  - y-interp: 16 matmuls per PSUM bank [128,448]; one bulk copy to bf16 SBUF.
    x-interp: 16 matmuls per PSUM bank [28,448]; the masked-BCE cross term is
    computed by the vector engine reading PSUM directly (no targets in SBUF).
  - host sums the 8 (bce_sum, num_pos) pairs into the global mean.
"""
import os
import sys
import types

sys.path.insert(0, "/opt/trn_rl_repo")

import numpy as np
import ml_dtypes
from contextlib import ExitStack

import concourse.bass as bass
import concourse.tile as tile
from concourse import mybir
from concourse.bass_utils import run_bass_kernel_spmd

# ---------------------------------------------------------------------------
# compatibility patches for this container's neuronxcc build
# ---------------------------------------------------------------------------


MAX_WAITS = 1
_applied = [False]


def apply_patches():
    if _applied[0]:
        return
    _applied[0] = True

    def _patched_cafs(self, sems):
        if not sems:
            return
        sem_nums = [s.num if hasattr(s, "num") else s for s in sems]
        for r in bass.compact_to_ranges(sem_nums):
            assert self._state.free_isdisjoint(r)
            self.gpsimd.dma_reset(r)  # drain w/ is_reset_sema resets the range
        self._state.prepend_free_semaphores(sem_nums)
        for poison_set in self._tile_sem_poison_stack:
            poison_set.update(sem_nums)

    bass.Bass.clear_and_free_semaphores = _patched_cafs


def split_excess_waits(nc):
    ctr = [0]
    for fn in nc.m.functions:
        for blk in fn.blocks:
            insts = list(blk.instructions)
            out = []
            changed = False
            for ins in insts:
                si = getattr(ins, "sync_info", None)
                if si is not None and si.on_wait and len(si.on_wait) > MAX_WAITS:
                    waits = list(si.on_wait)
                    excess, keep = waits[:-MAX_WAITS], waits[-MAX_WAITS:]
                    while excess:
                        chunk, excess = excess[:MAX_WAITS], excess[MAX_WAITS:]
                        ctr[0] += 1
                        out.append(mybir.InstNoOp(
                            name=f"I-waitsplit-{ctr[0]}",
                            engine=ins.engine,
                            bass_nofuse=True,
                            sync_info=mybir.SyncInfo(on_wait=chunk, on_update=[]),
                        ))
                    si.on_wait = keep
                    changed = True
                out.append(ins)
            if changed:
                blk.instructions = out
    return ctr[0]


F32 = mybir.dt.float32
BF16 = mybir.dt.bfloat16
I32 = mybir.dt.int32
AF = mybir.ActivationFunctionType
OP = mybir.AluOpType
ET = mybir.EngineType

P, G, H, W = 512, 200, 520, 704
PC = 64          # proposals per core
M = 28           # mask size
GH = G * H       # mask rows when flattened to [G*H, W]
PAD_ROWS = 8
SEG = 128        # crop window (rows and cols)
GRP = 16         # proposals per PSUM bank group
NG = PC // GRP   # 4 groups
CW = GRP * M     # 448 columns per group bank


def _floor_seq(nc, pool, x_ap, shape, name):
    """Exact floor for x>=0 on HW (f32->i32 conversion rounds to nearest).
    Returns (floor_f32_tile, floor_i32_tile)."""
    fi = pool.tile(shape, I32, tag=f"{name}_fi")
    nc.vector.tensor_copy(out=fi[:], in_=x_ap)
    ff = pool.tile(shape, F32, tag=f"{name}_ff")
    nc.vector.tensor_copy(out=ff[:], in_=fi[:])
    gt = pool.tile(shape, F32, tag=f"{name}_gt")
    nc.vector.tensor_sub(out=gt[:], in0=ff[:], in1=x_ap)
    nc.vector.tensor_scalar(out=gt[:], in0=gt[:], scalar1=0.0, scalar2=None,
                            op0=OP.is_gt)
    nc.vector.tensor_sub(out=ff[:], in0=ff[:], in1=gt[:])
    fi2 = pool.tile(shape, I32, tag=f"{name}_fi2")
    nc.vector.tensor_copy(out=fi2[:], in_=ff[:])
    return ff, fi2


def build(nc: bass.Bass):
    logitsT = nc.dram_tensor("logitsT", [M, PC * M], F32, kind="ExternalInput")
    props = nc.dram_tensor("props", [PC, 4], F32, kind="ExternalInput")
    gtbr = nc.dram_tensor("gtbr", [PC, 4 * G], F32, kind="ExternalInput")
    masks2 = nc.dram_tensor("masks2", [GH + PAD_ROWS, 2 * W], mybir.dt.uint8,
                            kind="ExternalInput")
    out = nc.dram_tensor("out", [2], F32, kind="ExternalOutput")
    scr_y0 = nc.dram_tensor("scr_y0", [PC, M], BF16)
    scr_wy = nc.dram_tensor("scr_wy", [PC, M], BF16)
    scr_x0 = nc.dram_tensor("scr_x0", [PC, M], BF16)
    scr_wx = nc.dram_tensor("scr_wx", [PC, M], BF16)
    scr_pos = nc.dram_tensor("scr_pos", [PC, 1], F32)
    scr_off = nc.dram_tensor("scr_off", [PC, 1], I32)

    with tile.TileContext(nc) as tc, ExitStack() as ctx:
        pool = ctx.enter_context(tc.tile_pool(name="sbuf", bufs=1))
        crop_pool = ctx.enter_context(tc.tile_pool(name="crops", bufs=20))
        psum_y = ctx.enter_context(tc.tile_pool(name="psum_y", bufs=2, space="PSUM"))
        psum_x = ctx.enter_context(tc.tile_pool(name="psum_x", bufs=2, space="PSUM"))
        psum_bc = ctx.enter_context(tc.tile_pool(name="psum_bc", bufs=2, space="PSUM"))

        # ---------- constants ----------
        iota_g_i = pool.tile([PC, G], I32)
        nc.gpsimd.iota(iota_g_i[:], pattern=[[1, G]], base=0, channel_multiplier=0)
        iota_g = pool.tile([PC, G], F32)
        nc.vector.tensor_copy(out=iota_g[:], in_=iota_g_i[:])
        grid_i = pool.tile([PC, M], I32)
        nc.gpsimd.iota(grid_i[:], pattern=[[1, M]], base=0, channel_multiplier=0)
        grid = pool.tile([PC, M], F32)
        nc.vector.tensor_copy(out=grid[:], in_=grid_i[:])
        nc.vector.tensor_scalar_add(out=grid[:], in0=grid[:], scalar1=0.5)
        ones1 = pool.tile([1, 128], F32)
        nc.vector.memset(ones1[:], 1.0)
        ones28 = pool.tile([M, 1], F32)
        nc.vector.memset(ones28[:], 1.0)
        # 3-row bf16 lhsT for the hat-matrix broadcast matmuls:
        # row0 = iota q (0..127), row1 = row2 = 1.0
        lhsT3 = pool.tile([3, 128], BF16)
        nc.vector.memset(lhsT3[:], 1.0)
        iotaq_i = pool.tile([1, 128], I32)
        nc.gpsimd.iota(iotaq_i[:], pattern=[[1, 128]], base=0, channel_multiplier=0)
        nc.vector.tensor_copy(out=lhsT3[0:1, :], in_=iotaq_i[:])

        # ---------- inputs ----------
        pr = pool.tile([PC, 4], F32)
        nc.sync.dma_start(out=pr[:], in_=props.ap())
        gb = pool.tile([PC, 4 * G], F32)
        nc.sync.dma_start(out=gb[:], in_=gtbr.ap())
        lg = pool.tile([M, PC * M], F32)
        nc.sync.dma_start(out=lg[:], in_=logitsT.ap())

        px1, py1, px2, py2 = (pr[:, i:i + 1] for i in range(4))
        gx1, gy1, gx2, gy2 = (gb[:, i * G:(i + 1) * G] for i in range(4))

        # ---------- stage 1: IoU [PC, G] ----------
        ltx = pool.tile([PC, G], F32)
        nc.vector.tensor_scalar_max(out=ltx[:], in0=gx1, scalar1=px1)
        iw = pool.tile([PC, G], F32)
        nc.vector.scalar_tensor_tensor(out=iw[:], in0=gx2, scalar=px2, in1=ltx[:],
                                       op0=OP.min, op1=OP.subtract)
        nc.vector.tensor_scalar_max(out=iw[:], in0=iw[:], scalar1=0.0)
        lty = pool.tile([PC, G], F32)
        nc.vector.tensor_scalar_max(out=lty[:], in0=gy1, scalar1=py1)
        ih = pool.tile([PC, G], F32)
        nc.vector.scalar_tensor_tensor(out=ih[:], in0=gy2, scalar=py2, in1=lty[:],
                                       op0=OP.min, op1=OP.subtract)
        nc.vector.tensor_scalar_max(out=ih[:], in0=ih[:], scalar1=0.0)
        inter = pool.tile([PC, G], F32)
        nc.vector.tensor_mul(out=inter[:], in0=iw[:], in1=ih[:])
        aw = pool.tile([PC, 1], F32)
        nc.vector.tensor_sub(out=aw[:], in0=px2, in1=px1)
        ah = pool.tile([PC, 1], F32)
        nc.vector.tensor_sub(out=ah[:], in0=py2, in1=py1)
        area_a = pool.tile([PC, 1], F32)
        nc.vector.tensor_mul(out=area_a[:], in0=aw[:], in1=ah[:])
        gw = pool.tile([PC, G], F32)
        nc.vector.tensor_sub(out=gw[:], in0=gx2, in1=gx1)
        gh_t = pool.tile([PC, G], F32)
        nc.vector.tensor_sub(out=gh_t[:], in0=gy2, in1=gy1)
        area_g = pool.tile([PC, G], F32)
        nc.vector.tensor_mul(out=area_g[:], in0=gw[:], in1=gh_t[:])
        denom = pool.tile([PC, G], F32)
        nc.vector.scalar_tensor_tensor(out=denom[:], in0=area_g[:], scalar=area_a[:],
                                       in1=inter[:], op0=OP.add, op1=OP.subtract)
        rec = pool.tile([PC, G], F32)
        nc.vector.reciprocal(out=rec[:], in_=denom[:])
        iou = pool.tile([PC, G], F32)
        nc.vector.tensor_mul(out=iou[:], in0=inter[:], in1=rec[:])

        # ---------- stage 2: match ----------
        miou = pool.tile([PC, 1], F32)
        nc.vector.tensor_reduce(out=miou[:], in_=iou[:], axis=mybir.AxisListType.X,
                                op=OP.max)
        pos = pool.tile([PC, 1], F32)
        nc.vector.tensor_scalar(out=pos[:], in0=miou[:], scalar1=0.3, scalar2=None,
                                op0=OP.is_gt)
        eq = pool.tile([PC, G], F32)
        nc.vector.tensor_scalar(out=eq[:], in0=iou[:], scalar1=miou[:], scalar2=None,
                                op0=OP.is_ge)
        masked = pool.tile([PC, G], F32)
        nc.vector.scalar_tensor_tensor(out=masked[:], in0=eq[:], scalar=-1024.0,
                                       in1=iota_g[:], op0=OP.mult, op1=OP.add)
        midxf = pool.tile([PC, 1], F32)
        nc.vector.tensor_reduce(out=midxf[:], in_=masked[:], axis=mybir.AxisListType.X,
                                op=OP.min)
        nc.vector.tensor_scalar_add(out=midxf[:], in0=midxf[:], scalar1=1024.0)
        onehot = pool.tile([PC, G], F32)
        nc.vector.tensor_scalar(out=onehot[:], in0=iota_g[:], scalar1=midxf[:],
                                scalar2=None, op0=OP.is_equal)
        mscr = pool.tile([PC, G], F32)
        mb = pool.tile([PC, 4], F32)
        for c, gcomp in enumerate((gx1, gy1, gx2, gy2)):
            nc.vector.tensor_mul(out=mscr[:], in0=onehot[:], in1=gcomp)
            nc.vector.tensor_reduce(out=mb[:, c:c + 1], in_=mscr[:],
                                    axis=mybir.AxisListType.X, op=OP.add)
        midx_i = pool.tile([PC, 1], I32)
        nc.vector.tensor_copy(out=midx_i[:], in_=midxf[:])  # exact integer value

        # ---------- stage 3: crop params ----------
        _, bi = _floor_seq(nc, pool, mb[:], [PC, 4], "bi")   # trunc(mb), mb>=0
        x1c = pool.tile([PC, 1], I32)
        nc.vector.tensor_scalar(out=x1c[:], in0=bi[:, 0:1], scalar1=0, scalar2=W - 1,
                                op0=OP.max, op1=OP.min)
        y1c = pool.tile([PC, 1], I32)
        nc.vector.tensor_scalar(out=y1c[:], in0=bi[:, 1:2], scalar1=0, scalar2=H - 1,
                                op0=OP.max, op1=OP.min)
        x2t = pool.tile([PC, 1], I32)
        nc.vector.tensor_scalar(out=x2t[:], in0=bi[:, 2:3], scalar1=W, scalar2=None,
                                op0=OP.min)
        x1p1 = pool.tile([PC, 1], I32)
        nc.vector.tensor_scalar_add(out=x1p1[:], in0=x1c[:], scalar1=1)
        x2c = pool.tile([PC, 1], I32)
        nc.vector.tensor_max(out=x2c[:], in0=x1p1[:], in1=x2t[:])
        y2t = pool.tile([PC, 1], I32)
        nc.vector.tensor_scalar(out=y2t[:], in0=bi[:, 3:4], scalar1=H, scalar2=None,
                                op0=OP.min)
        y1p1 = pool.tile([PC, 1], I32)
        nc.vector.tensor_scalar_add(out=y1p1[:], in0=y1c[:], scalar1=1)
        y2c = pool.tile([PC, 1], I32)
        nc.vector.tensor_max(out=y2c[:], in0=y1p1[:], in1=y2t[:])
        cw_i = pool.tile([PC, 1], I32)
        nc.vector.tensor_sub(out=cw_i[:], in0=x2c[:], in1=x1c[:])
        ch_i = pool.tile([PC, 1], I32)
        nc.vector.tensor_sub(out=ch_i[:], in0=y2c[:], in1=y1c[:])
        cw_f = pool.tile([PC, 1], F32)
        nc.vector.tensor_copy(out=cw_f[:], in_=cw_i[:])
        ch_f = pool.tile([PC, 1], F32)
        nc.vector.tensor_copy(out=ch_f[:], in_=ch_i[:])
        ox = pool.tile([PC, 1], I32)
        nc.vector.tensor_scalar(out=ox[:], in0=x1c[:], scalar1=W - SEG, scalar2=None,
                                op0=OP.min)
        oy = pool.tile([PC, 1], I32)
        nc.vector.tensor_scalar(out=oy[:], in0=y1c[:], scalar1=H - SEG, scalar2=None,
                                op0=OP.min)
        dx_i = pool.tile([PC, 1], I32)
        nc.vector.tensor_sub(out=dx_i[:], in0=x1c[:], in1=ox[:])
        dx_f = pool.tile([PC, 1], F32)
        nc.vector.tensor_copy(out=dx_f[:], in_=dx_i[:])
        dy_i = pool.tile([PC, 1], I32)
        nc.vector.tensor_sub(out=dy_i[:], in0=y1c[:], in1=oy[:])
        dy_f = pool.tile([PC, 1], F32)
        nc.vector.tensor_copy(out=dy_f[:], in_=dy_i[:])
        # crop DMA byte offset: ((midx*H + oy)*W + ox) * 2  (i32 exact)
        offl = pool.tile([PC, 1], I32)
        nc.vector.tensor_scalar_mul(out=offl[:], in0=midx_i[:], scalar1=H)
        nc.vector.tensor_add(out=offl[:], in0=offl[:], in1=oy[:])
        nc.vector.tensor_scalar_mul(out=offl[:], in0=offl[:], scalar1=W)
        nc.vector.tensor_add(out=offl[:], in0=offl[:], in1=ox[:])
        nc.vector.tensor_scalar_mul(out=offl[:], in0=offl[:], scalar1=2)
        nc.sync.dma_start(out=scr_off.ap(), in_=offl[:])
        offrow = pool.tile([1, PC], I32)
        nc.sync.dma_start(out=offrow[:],
                          in_=scr_off.ap().rearrange("a b -> (a b)").unsqueeze(0))

        # ---------- stage 4: sample coords -> (floor, frac) rows ----------
        def coords(cf, df, scr0, scrw, name):
            cm1 = pool.tile([PC, 1], F32, tag=f"{name}_cm1")
            nc.vector.tensor_scalar_add(out=cm1[:], in0=cf[:], scalar1=-1.0)
            cd = pool.tile([PC, 1], F32, tag=f"{name}_cd")
            nc.vector.tensor_scalar_mul(out=cd[:], in0=cf[:], scalar1=1.0 / M)
            s = pool.tile([PC, M], F32, tag=f"{name}_s")
            nc.vector.tensor_scalar(out=s[:], in0=grid[:], scalar1=cd[:],
                                    scalar2=-0.5, op0=OP.mult, op1=OP.add)
            nc.vector.tensor_scalar(out=s[:], in0=s[:], scalar1=0.0, scalar2=cm1[:],
                                    op0=OP.max, op1=OP.min)
            nc.vector.tensor_scalar_add(out=s[:], in0=s[:], scalar1=df[:])
            s0f, _ = _floor_seq(nc, pool, s[:], [PC, M], f"{name}_fl")
            # negated bf16 copies (rhs rows are -floor, -frac)
            n0 = pool.tile([PC, M], BF16, tag=f"{name}_n0")
            nc.vector.tensor_scalar_mul(out=n0[:], in0=s0f[:], scalar1=-1.0)
            wfr = pool.tile([PC, M], F32, tag=f"{name}_wf")
            nc.vector.tensor_sub(out=wfr[:], in0=s[:], in1=s0f[:])
            nw = pool.tile([PC, M], BF16, tag=f"{name}_nw")
            nc.vector.tensor_scalar_mul(out=nw[:], in0=wfr[:], scalar1=-1.0)
            nc.sync.dma_start(out=scr0.ap(), in_=n0[:])
            nc.sync.dma_start(out=scrw.ap(), in_=nw[:])
            rhs3 = pool.tile([3, PC * M], BF16, tag=f"{name}_rhs3")
            nc.vector.memset(rhs3[:], 1.0)
            nc.sync.dma_start(
                out=rhs3[1:2, :],
                in_=scr0.ap().rearrange("a b -> (a b)").unsqueeze(0))
            nc.sync.dma_start(
                out=rhs3[2:3, :],
                in_=scrw.ap().rearrange("a b -> (a b)").unsqueeze(0))
            return rhs3

        rhsY = coords(ch_f, dy_f, scr_y0, scr_wy, "sy")
        rhsX = coords(cw_f, dx_f, scr_x0, scr_wx, "sx")
        nc.sync.dma_start(out=scr_pos.ap(), in_=pos[:])
        pos_row = pool.tile([1, PC], F32)
        nc.sync.dma_start(out=pos_row[:],
                          in_=scr_pos.ap().rearrange("a b -> (a b)").unsqueeze(0))

        # ---------- stage 5: hat interp matrices [128, PC*M] bf16 ----------
        def hat_matrix(rhs3, name):
            rt = pool.tile([128, PC * M], BF16, tag=f"{name}_rt")
            tmp = pool.tile([128, PC * M], BF16, tag=f"{name}_tmp")
            for c in range(4):
                dps = psum_bc.tile([128, CW], F32, tag="bc")
                nc.tensor.matmul(out=dps[:], lhsT=lhsT3[:],
                                 rhs=rhs3[:, c * CW:(c + 1) * CW],
                                 start=True, stop=True)
                nc.scalar.activation(out=tmp[:, c * CW:(c + 1) * CW], in_=dps[:],
                                     func=AF.Abs)
                nc.scalar.activation(out=rt[:, c * CW:(c + 1) * CW],
                                     in_=tmp[:, c * CW:(c + 1) * CW],
                                     func=AF.Relu, scale=-1.0, bias=1.0)
            return rt

        ryt = hat_matrix(rhsY, "ry")
        rxt = hat_matrix(rhsX, "rx")

        # ---------- stage 6: pos broadcast + logit masking ----------
        posbc_ps = psum_bc.tile([M, PC], F32, tag="bc")
        nc.tensor.matmul(out=posbc_ps[:], lhsT=ones1[0:1, 0:M], rhs=pos_row[:],
                         start=True, stop=True)
        pos_bc = pool.tile([M, PC], F32)
        nc.scalar.copy(out=pos_bc[:], in_=posbc_ps[:])
        lm = pool.tile([M, PC * M], F32)
        nc.vector.tensor_tensor(
            out=lm[:].rearrange("n (p m) -> n p m", p=PC),
            in0=lg[:].rearrange("n (p m) -> n p m", p=PC),
            in1=pos_bc[:].unsqueeze(2).to_broadcast([M, PC, M]),
            op=OP.mult)

        # ---------- stage 7: softplus branch (independent of crops) ----------
        sabs = pool.tile([M, PC * M], F32)
        nc.scalar.activation(out=sabs[:], in_=lg[:], func=AF.Abs)
        nc.scalar.activation(out=sabs[:], in_=sabs[:], func=AF.Exp, scale=-1.0)
        nc.scalar.activation(out=sabs[:], in_=sabs[:], func=AF.Ln, bias=1.0)
        srelu = pool.tile([M, PC * M], F32)
        nc.scalar.activation(out=srelu[:], in_=lg[:], func=AF.Relu)
        sp = pool.tile([M, PC * M], F32)
        nc.vector.tensor_add(out=sp[:], in0=sabs[:], in1=srelu[:])
        scr7 = pool.tile([M, PC * M], F32)
        nc.vector.tensor_tensor(
            out=scr7[:].rearrange("n (p m) -> n p m", p=PC),
            in0=sp[:].rearrange("n (p m) -> n p m", p=PC),
            in1=pos_bc[:].unsqueeze(2).to_broadcast([M, PC, M]),
            op=OP.mult)
        spm = pool.tile([M, 1], F32)
        nc.vector.tensor_reduce(out=spm[:], in_=scr7[:],
                                axis=mybir.AxisListType.X, op=OP.add)

        # ---------- stage 8: crop fetch + y/x interp + cross term ----------
        base_ap = masks2.ap()[0:SEG, 0:2 * SEG]   # [[2W,128],[1,256]] u8
        dma_engs = [(nc.sync, ET.SP, 0, 32), (nc.gpsimd, ET.Pool, 32, 32)]
        eng_of = {}
        offvals = [None] * PC
        for eng, et, base, cnt in dma_engs:
            for p in range(base, base + cnt):
                eng_of[p] = eng
            for blk in range(0, cnt, 8):
                lo = base + blk
                _, vs = nc.values_load_multi_w_load_instructions(
                    offrow[0:1, lo:lo + 8], engines=(et,),
                    min_val=0, max_val=(GH + PAD_ROWS - SEG) * 2 * W,
                    skip_runtime_bounds_check=True)
                for k in range(8):
                    offvals[lo + k] = vs[k]

        crops = [None] * PC
        for p in range(PC):
            crop = crop_pool.tile([SEG, 2 * SEG], mybir.dt.uint8, tag="crop")
            cap = bass.AP(tensor=base_ap.tensor, offset=offvals[p],
                          ap=base_ap.ap)
            eng_of[p].dma_start(out=crop[:], in_=cap)
            crops[p] = crop

        cross_cols = pool.tile([M, NG], F32)
        for g in range(NG):
            t1t_ps = psum_y.tile([SEG, CW], F32, tag="t1t")
            for i in range(GRP):
                pp = g * GRP + i
                nc.tensor.matmul(out=t1t_ps[:, i * M:(i + 1) * M],
                                 lhsT=crops[pp][:].bitcast(BF16),
                                 rhs=ryt[:, pp * M:(pp + 1) * M],
                                 start=True, stop=True)
            t1t_sb = crop_pool.tile([SEG, CW], BF16, tag="t1sb")
            nc.scalar.copy(out=t1t_sb[:], in_=t1t_ps[:])
            tg_ps = psum_x.tile([M, CW], F32, tag="tg")
            for i in range(GRP):
                pp = g * GRP + i
                nc.tensor.matmul(out=tg_ps[:, i * M:(i + 1) * M],
                                 lhsT=rxt[:, pp * M:(pp + 1) * M],
                                 rhs=t1t_sb[:, i * M:(i + 1) * M],
                                 start=True, stop=True)
            # cross term: sum(lm_g * targets_g) straight from PSUM
            xs = crop_pool.tile([M, CW], F32, tag="xs")
            nc.vector.tensor_mul(out=xs[:], in0=lm[:, g * CW:(g + 1) * CW],
                                 in1=tg_ps[:])
            nc.vector.tensor_reduce(out=cross_cols[:, g:g + 1], in_=xs[:],
                                    axis=mybir.AxisListType.X, op=OP.add)

        # ---------- stage 9: reduce to (bce_sum, num_pos) ----------
        cross = pool.tile([M, 1], F32)
        nc.vector.tensor_reduce(out=cross[:], in_=cross_cols[:],
                                axis=mybir.AxisListType.X, op=OP.add)
        bce_col = pool.tile([M, 1], F32)
        nc.vector.tensor_sub(out=bce_col[:], in0=spm[:], in1=cross[:])
        tot_ps = psum_bc.tile([1, 1], F32, tag="bc")
        nc.tensor.matmul(out=tot_ps[:], lhsT=ones28[:], rhs=bce_col[:],
                         start=True, stop=True)
        out_sb = pool.tile([1, 2], F32)
        nc.scalar.copy(out=out_sb[:, 0:1], in_=tot_ps[:])
        nc.vector.tensor_reduce(out=out_sb[:, 1:2], in_=pos_row[:],
                                axis=mybir.AxisListType.X, op=OP.add)
        nc.sync.dma_start(out=out.ap().unsqueeze(0), in_=out_sb[:])

    return nc


def prep_inputs(mask_logits, proposals, gt_boxes, gt_masks, gt_labels=None):
    """Full inputs -> list of 8 per-core input maps."""
    mask_logits = np.asarray(mask_logits, np.float32)
    proposals = np.asarray(proposals, np.float32)
    gt_boxes = np.asarray(gt_boxes, np.float32)
    gt_masks = np.asarray(gt_masks, np.float32)
    gtbr = np.tile(gt_boxes.T.reshape(1, 4 * G), (PC, 1)).astype(np.float32)
    gtbr = np.ascontiguousarray(gtbr)
    m2 = np.zeros((GH + PAD_ROWS, W), ml_dtypes.bfloat16)
    m2[:GH] = gt_masks.reshape(GH, W).astype(ml_dtypes.bfloat16)
    m2 = m2.view(np.uint8)
    maps = []
    for c in range(8):
        sl = slice(c * PC, (c + 1) * PC)
        L = mask_logits[sl, 1]                      # [PC, M(m=y), M(n=x)]
        logitsT = np.ascontiguousarray(L.transpose(2, 0, 1).reshape(M, PC * M))
        maps.append({
            "logitsT": logitsT,
            "props": np.ascontiguousarray(proposals[sl]),
            "gtbr": gtbr,
            "masks2": m2,
        })
    return maps


def combine_outputs(outs):
    """outs: list of 8 np arrays [2] -> scalar float32 loss."""
    s = np.float32(0.0)
    n = np.float32(0.0)
    for o in outs:
        s = np.float32(s + np.float32(o[0]))
        n = np.float32(n + np.float32(o[1]))
    denom = np.float32(max(n, np.float32(1.0)) * np.float32(M * M))
    loss = np.float32(s / denom)
    return np.float32(loss if n > 0 else 0.0)


# ---------------------------------------------------------------------------
# public entry point
# ---------------------------------------------------------------------------
LAST_EXEC_NS = None
_BUILT = None


def _get_program():
    global _BUILT
    if _BUILT is None:
        apply_patches()
        nc = bass.Bass("TRN2", debug=False)
        build(nc)
        split_excess_waits(nc)
        _BUILT = nc
    return _BUILT


def kernel(mask_logits, proposals, gt_boxes, gt_masks, gt_labels=None, **_):
    global LAST_EXEC_NS
    nc = _get_program()
    maps = prep_inputs(mask_logits, proposals, gt_boxes, gt_masks, gt_labels)
    trace = os.environ.get("BASSKERNEL_TRACE", "0") == "1"
    if trace:
        try:
            from trn_agent_boot.trn_boot import _ntff_profile_via_ctypes
            hook = _ntff_profile_via_ctypes("/opt/axon/libaxon_pjrt.so")
            m = types.ModuleType("antenv.axon_hooks")
            m.get_axon_ntff_profile_hook = lambda: hook
            sys.modules["antenv.axon_hooks"] = m
        except Exception:
            trace = False
    res = run_bass_kernel_spmd(nc, maps, core_ids=list(range(8)), trace=trace)
    LAST_EXEC_NS = res.exec_time_ns
    outs = [res.results[c]["out"] for c in range(8)]
    return combine_outputs(outs)
